# revision 22
# baseline (speedup 1.0000x reference)
"""Trainium2 Bass kernel for nn_ActorCritic (GAT actor-critic over B=16384
fully-connected 6-node graphs), data-parallel over 8 NeuronCores.

v3: fp8 DoubleRow (K=256/pass) for the Wh and head GEMMs; embed / zn /
aggregation stay bf16 (fp8 there costs accuracy and buys no PE time).
phase_a is split into 4 pipeline stages so every DMA's producer finished
>=1 super earlier -- the DRAM round trips (elr transpose, attention
scatter) no longer stall the DMA queues.

Self-contained: hardcodes all shapes; imports only the system concourse repo.
"""
import sys

if "/opt/trn_rl_repo" not in sys.path:
    sys.path.insert(0, "/opt/trn_rl_repo")

import numpy as np
import ml_dtypes

import concourse.bass as bass
import concourse.mybir as mybir
import concourse.tile as tile
from concourse import bacc

BF16 = mybir.dt.bfloat16
F32 = mybir.dt.float32
FP8 = mybir.dt.float8e4
I16 = mybir.dt.int16
DR = mybir.MatmulPerfMode.DoubleRow

# problem constants
B, A, DOBS, D, H, HID, NACT = 16384, 6, 128, 250, 3, 256, 5
SLOPE = 0.2
NCORES = 8
G_PC = B // NCORES            # 2048 real graphs / core
GSUP = 84                     # graphs per super-block
SUPER = GSUP * A              # 504 nodes per super-block
NSUP_FULL = 25                # supers per core (pad 2048 -> 2100 graphs)
CHUNK = 126                   # nodes per aggregation chunk (21 graphs)
NCHUNK = SUPER // CHUNK       # 4
DIST = 6                      # pipeline distance between stage A1 and B
XT_PREF = 2                   # xt prefetch distance (supers)

# scale plan (stored = scale * true):
#   x: 1 (bf16)   wemb: 8 (bf16)   hT = 8*h (fp8)   wg: 32 (fp8)
#   Wh psum / wsb bf16 = 256*Wh    at4 = attn (bf16)
#   gat psum = 256*gat_sum         feat gat subtiles = 4*gat_sum (fp8)
#   W1 top *4, W1 bot *(8/3) (fp8) -> heads psum = 32*y1
#   a1 = 32*relu (bf16, bias*32)   w2 raw (bf16) -> zn psum = 32*z
#   vlr: 64 (fp8) -> elr psum = 512*elr (de-scaled in the exp)


def build(nsup=NSUP_FULL):
    """Trace + compile the per-core Bass graph. Returns compiled Bacc."""
    npc = nsup * SUPER  # padded nodes per core
    nc = bacc.Bacc("TRN2", target_bir_lowering=False, debug=False,
                   num_devices=NCORES)

    # ---- DRAM parameters ----
    xt_ext = nc.dram_tensor("xt", [DOBS, npc + 2], BF16, kind="ExternalInput")
    wemb_ext = nc.dram_tensor("wemb", [DOBS, 256], BF16, kind="ExternalInput")
    wg01_ext = nc.dram_tensor("wg01", [128, 2, 512], FP8, kind="ExternalInput")
    wg2_ext = nc.dram_tensor("wg2", [128, 2, 256], FP8, kind="ExternalInput")
    vlrp_ext = nc.dram_tensor("vlrp", [128, 2, 16], FP8, kind="ExternalInput")
    w1p_ext = nc.dram_tensor("w1p", [128, 4, 256], FP8, kind="ExternalInput")
    v1p_ext = nc.dram_tensor("v1p", [128, 4, 256], FP8, kind="ExternalInput")
    w2p_ext = nc.dram_tensor("w2p", [128, 2, 8], BF16, kind="ExternalInput")
    v2p_ext = nc.dram_tensor("v2p", [128, 2, 1], BF16, kind="ExternalInput")
    bias_ext = nc.dram_tensor("biases", [128, 8], F32, kind="ExternalInput")
    b2c_ext = nc.dram_tensor("b2cat", [128, 24], F32, kind="ExternalInput")
    sidx_ext = nc.dram_tensor("sidx", [128, 80], I16, kind="ExternalInput")
    out_ext = nc.dram_tensor("out", [npc, 6], F32, kind="ExternalOutput")

    # DRAM scratch for cross-layout reshapes. Global layouts: the attention
    # stats stages run on 126-graph tiles (full lanes) decoupled from the
    # 84-graph super cadence; DRAM is the granularity adapter.
    ntiles = (nsup * GSUP + 125) // 126
    ngpad = ntiles * 126
    elrd = nc.dram_tensor("elrd", [6, ngpad * A], F32)
    natd = nc.dram_tensor("natd", [ngpad, 108], BF16)

    with tile.TileContext(nc) as tc:
        # ---- persistent SBUF: weights/constants ----
        wemb_sb = nc.alloc_sbuf_tensor("wemb_sb", [DOBS, 256], BF16)
        wg01_sb = nc.alloc_sbuf_tensor("wg01_sb", [128, 2, 512], FP8)
        wg2_sb = nc.alloc_sbuf_tensor("wg2_sb", [128, 2, 256], FP8)
        vlrp_sb = nc.alloc_sbuf_tensor("vlrp_sb", [128, 2, 16], FP8)
        w1p_sb = nc.alloc_sbuf_tensor("w1p_sb", [128, 4, 256], FP8)
        v1p_sb = nc.alloc_sbuf_tensor("v1p_sb", [128, 4, 256], FP8)
        w2p_sb = nc.alloc_sbuf_tensor("w2p_sb", [128, 2, 8], BF16)
        v2p_sb = nc.alloc_sbuf_tensor("v2p_sb", [128, 2, 1], BF16)
        bias_sb = nc.alloc_sbuf_tensor("bias_sb", [128, 8], F32)
        b2c_sb = nc.alloc_sbuf_tensor("b2c_sb", [128, 24], F32)
        sidx_sb = nc.alloc_sbuf_tensor("sidx_sb", [128, 80], I16)

        nc.scalar.dma_start(wemb_sb[:], wemb_ext[:])
        nc.scalar.dma_start(bias_sb[:], bias_ext[:])
        nc.scalar.dma_start(vlrp_sb[:], vlrp_ext[:])
        nc.scalar.dma_start(sidx_sb[:], sidx_ext[:])

        deferred = [(wg01_sb[:], wg01_ext[:]), (wg2_sb[:], wg2_ext[:]),
                    (w1p_sb[:], w1p_ext[:]), (v1p_sb[:], v1p_ext[:]),
                    (w2p_sb[:], w2p_ext[:]), (v2p_sb[:], v2p_ext[:]),
                    (b2c_sb[:], b2c_ext[:])]
        _pump_i = [0]

        def pump_preloads(n):
            engines = [nc.sync, nc.scalar, nc.gpsimd]
            for _ in range(n):
                if not deferred:
                    return
                dst, srcd = deferred.pop(0)
                engines[_pump_i[0] % 3].dma_start(dst, srcd)
                _pump_i[0] += 1

        # ---- pools ----
        import contextlib
        ctx = contextlib.ExitStack()
        with ctx:
            xpool = ctx.enter_context(tc.tile_pool(name="xp", bufs=4))
            hpool = ctx.enter_context(tc.tile_pool(name="hp", bufs=DIST + 1))
            whpool = ctx.enter_context(tc.tile_pool(name="whp", bufs=4))
            a1pool = ctx.enter_context(tc.tile_pool(name="a1p", bufs=8))
            apool = ctx.enter_context(tc.tile_pool(name="ap", bufs=DIST + 1))
            spool = ctx.enter_context(tc.tile_pool(name="sp", bufs=4))
            opool = ctx.enter_context(tc.tile_pool(name="op", bufs=3))
            # PSUM: 8 banks total; tags: emb 2 + elr 1 + big2 3 + gat 2
            ppool = ctx.enter_context(tc.tile_pool(name="pp", bufs=2, space="PSUM"))

            xt_tiles = {}

            def load_xt(s):
                n0 = s * SUPER
                xt = xpool.tile([DOBS, SUPER + 2], BF16, tag="xt")
                nc.sync.dma_start(xt[:], xt_ext[:, n0:n0 + SUPER + 2])
                xt_tiles[s] = xt

            def phase_a1(s):
                """embed + elr matmul + elr -> DRAM."""
                if s + XT_PREF < nsup:
                    load_xt(s + XT_PREF)
                xt = xt_tiles.pop(s)
                h_ps0 = ppool.tile([128, SUPER + 2], F32, tag="emb", bufs=2)
                h_ps1 = ppool.tile([128, SUPER + 2], F32, tag="emb", bufs=2)
                nc.tensor.matmul(h_ps0[:], wemb_sb[:, 0:128], xt[:],
                                 start=True, stop=True)
                nc.tensor.matmul(h_ps1[:], wemb_sb[:, 128:256], xt[:],
                                 start=True, stop=True)
                # hT: fp8 feat tile; subtiles 0,1 = 8*h halves (2,3 = gat later)
                hT = hpool.tile([128, 4, 512], FP8, tag="hT")
                nc.vector.tensor_scalar(
                    hT[:, 0, 0:506], h_ps0[:, 0:506],
                    bias_sb[:, 0:1], 0.0,
                    mybir.AluOpType.add, mybir.AluOpType.max)
                nc.scalar.activation(
                    hT[:, 1, 0:506], h_ps1[:, 0:506],
                    mybir.ActivationFunctionType.Relu,
                    bias=bias_sb[:, 1:2])
                # el/er (fp8 DoubleRow, K=256 in one pass)
                elr_ps = ppool.tile([6, SUPER + 2], F32, tag="elr", bufs=1)
                nc.tensor.matmul(elr_ps[:, 0:SUPER], vlrp_sb[:, :, 0:6],
                                 hT[:, 0:2, 0:SUPER],
                                 start=True, stop=True, perf_mode=DR)
                elr_sb = spool.tile([6, SUPER], F32, tag="elr_sb")
                nc.scalar.copy(elr_sb[:], elr_ps[:, 0:SUPER])
                nc.sync.dma_start(elrd[:, s * SUPER:(s + 1) * SUPER], elr_sb[:])
                pump_preloads(5)
                return {"hT": hT}

            tstates = {}

            def tile_a2(t):
                """read elr back transposed to graph-major, 126-graph tile."""
                elrg = spool.tile([126, 36], F32, tag="elrg")
                src = bass.AP(tensor=elrd[:].tensor, offset=t * 126 * 6,
                              ap=[[6, 126], [ngpad * A, 6], [1, 6]])
                nc.sync.dma_start(elrg[:].rearrange("p (r i) -> p r i", i=6), src)
                tstates[t] = elrg

            def tile_a3(t):
                """attention stats (graph-major, 126 lanes) + natd -> DRAM."""
                elrg = tstates.pop(t)
                e_t = spool.tile([126, 108], F32, tag="e_t")
                in0 = (elrg[:, 0:18].rearrange("p (h i) -> p h i", i=6)
                       .unsqueeze(3).broadcast_to((126, 3, 6, 6)))
                in1 = (elrg[:, 18:36].rearrange("p (h j) -> p h j", j=6)
                       .unsqueeze(2).broadcast_to((126, 3, 6, 6)))
                ev = e_t[:].rearrange("p (h i j) -> p h i j", i=6, j=6)
                nc.vector.tensor_tensor(ev, in0, in1, mybir.AluOpType.add)
                w_t = spool.tile([126, 108], F32, tag="w_t")
                lk = spool.tile([126, 108], F32, tag="lk")
                nc.vector.scalar_tensor_tensor(lk[:], e_t[:], SLOPE, e_t[:],
                                               mybir.AluOpType.mult,
                                               mybir.AluOpType.max)
                # elr psum carries 512x scale; fold 1/512 into the exp
                nc.scalar.activation(w_t[:], lk[:],
                                     mybir.ActivationFunctionType.Exp,
                                     scale=1.0 / 512.0)
                s_t = spool.tile([126, 18], F32, tag="s_t")
                nc.vector.tensor_reduce(
                    s_t[:], w_t[:].rearrange("p (hi j) -> p hi j", j=6),
                    mybir.AxisListType.X, mybir.AluOpType.add)
                r_t = spool.tile([126, 18], F32, tag="r_t")
                nc.vector.reciprocal(r_t[:], s_t[:])
                natg = spool.tile([126, 112], BF16, tag="natg")
                nout = natg[:, 0:108].rearrange("p (j h i) -> p h i j", h=3, i=6)
                nin0 = w_t[:].rearrange("p (h i j) -> p h i j", i=6, j=6)
                nin1 = (r_t[:].rearrange("p (h i) -> p h i", i=6)
                        .unsqueeze(3).broadcast_to((126, 3, 6, 6)))
                nc.vector.tensor_tensor(nout, nin0, nin1, mybir.AluOpType.mult)
                nc.gpsimd.dma_start(natd[t * 126:(t + 1) * 126], natg[:, 0:108])

            def phase_a4(s, st):
                """node-major gather + block-diag scatter."""
                dataN4 = spool.tile([128, 80], BF16, tag="dataN")
                engs = [nc.sync, nc.gpsimd, nc.sync, nc.gpsimd]
                for k in range(NCHUNK):
                    srcn = bass.AP(tensor=natd[:].tensor,
                                   offset=(s * GSUP + 21 * k) * 108,
                                   ap=[[108, 21], [18, 6], [1, 18]])
                    engs[k].dma_start(dataN4[0:CHUNK, 20 * k:20 * k + 18], srcn)
                at4 = apool.tile([128, 4 * 384], BF16, tag="atall")
                nc.gpsimd.local_scatter(at4[:], dataN4[:], sidx_sb[:],
                                        channels=128, num_elems=4 * 384,
                                        num_idxs=80)
                st["at4"] = at4

            def phase_b(st):
                hT, at4 = st["hT"], st["at4"]
                s = st["s"]
                n0 = s * SUPER
                # ---------- per-chunk: Wh (fp8 DoubleRow) + aggregation ----
                gatL = ppool.tile([128, SUPER], F32, tag="gat", bufs=2)
                gatU = ppool.tile([128, SUPER], F32, tag="gat", bufs=2)
                wh_tiles = [None] * NCHUNK

                def wh_chunk(k):
                    c0 = k * CHUNK
                    whA = ppool.tile([126, 512], F32, tag="big2", bufs=3)
                    nc.tensor.matmul(whA[:], hT[:, 0:2, c0:c0 + CHUNK],
                                     wg01_sb[:], start=True, stop=True,
                                     perf_mode=DR)
                    whB = ppool.tile([126, 256], F32, tag="big2", bufs=3)
                    nc.tensor.matmul(whB[:], hT[:, 0:2, c0:c0 + CHUNK],
                                     wg2_sb[:], start=True, stop=True,
                                     perf_mode=DR)
                    wsb01 = whpool.tile([126, 512], BF16, tag="wh01")
                    nc.vector.tensor_copy(wsb01[:], whA[:])
                    wsb2 = whpool.tile([126, 256], BF16, tag="wh2")
                    nc.vector.tensor_copy(wsb2[:], whB[:])
                    wh_tiles[k] = (wsb01, wsb2)

                def gat_chunk(k):
                    c0 = k * CHUNK
                    wsb01, wsb2 = wh_tiles[k]
                    # hi halves carry 6 zero pad cols so M=128 everywhere
                    wh_t = [(wsb01, 0, 128), (wsb01, 256, 384), (wsb2, 0, 128)]
                    for h in range(H):
                        tl, lo, _ = wh_t[h]
                        nc.tensor.matmul(
                            gatL[:, c0:c0 + CHUNK],
                            tl[0:CHUNK, lo:lo + 128],
                            at4[0:CHUNK, 384 * k + 128 * h:384 * k + 128 * h + CHUNK],
                            start=(h == 0), stop=(h == 2))
                    for h in range(H):
                        tl, _, hi = wh_t[h]
                        nc.tensor.matmul(
                            gatU[:, c0:c0 + CHUNK],
                            tl[0:CHUNK, hi:hi + 128],
                            at4[0:CHUNK, 384 * k + 128 * h:384 * k + 128 * h + CHUNK],
                            start=(h == 0), stop=(h == 2))

                wh_chunk(0)
                wh_chunk(1)
                gat_chunk(0)
                wh_chunk(2)
                gat_chunk(1)
                wh_chunk(3)
                gat_chunk(2)
                gat_chunk(3)
                # gat psum = 256*gat_sum; store 4*gat_sum into feat subtiles
                nc.vector.tensor_scalar(hT[:, 2, 0:SUPER], gatL[:, 0:SUPER],
                                        1.0 / 64.0, 0.0,
                                        mybir.AluOpType.mult,
                                        mybir.AluOpType.add)
                nc.scalar.activation(hT[:, 3, 0:SUPER], gatU[:, 0:SUPER],
                                     mybir.ActivationFunctionType.Copy,
                                     scale=1.0 / 64.0)

                # ---------- heads (fp8 DoubleRow, 2 K-passes of 256) -------
                a1_sb, c1_sb = [], []
                for which, (wp, bcol, dst) in enumerate(
                        [(w1p_sb, 2, a1_sb), (v1p_sb, 4, c1_sb)]):
                    for m in range(2):
                        ps = ppool.tile([128, SUPER], F32, tag="big2", bufs=3)
                        nc.tensor.matmul(ps[:],
                                         wp[:, 0:2, 128 * m:128 * m + 128],
                                         hT[:, 0:2, 0:SUPER],
                                         start=True, stop=False, perf_mode=DR)
                        nc.tensor.matmul(ps[:],
                                         wp[:, 2:4, 128 * m:128 * m + 128],
                                         hT[:, 2:4, 0:SUPER],
                                         start=False, stop=True, perf_mode=DR)
                        sb = a1pool.tile([128, SUPER + 4], BF16, tag="a1")
                        bap = bias_sb[:, bcol + m:bcol + m + 1]
                        if (which, m) == (0, 0):
                            nc.vector.tensor_scalar(
                                sb[:, 0:SUPER], ps[:], bap, 0.0,
                                mybir.AluOpType.add, mybir.AluOpType.max)
                        else:
                            nc.scalar.activation(
                                sb[:, 0:SUPER], ps[:],
                                mybir.ActivationFunctionType.Relu, bias=bap)
                        dst.append(sb)
                # ---------- zn (bf16) ----------
                zn_ps = ppool.tile([128, 24], F32, tag="gat", bufs=2)
                for k in range(NCHUNK):
                    c0 = k * CHUNK
                    for kt in range(2):
                        nc.tensor.matmul(zn_ps[:, 6 * k:6 * k + NACT],
                                         a1_sb[kt][:, c0:c0 + 128],
                                         w2p_sb[:, kt, 0:NACT],
                                         start=(kt == 0), stop=(kt == 1))
                    for kt in range(2):
                        nc.tensor.matmul(zn_ps[:, 6 * k + NACT:6 * k + 6],
                                         c1_sb[kt][:, c0:c0 + 128],
                                         v2p_sb[:, kt, :],
                                         start=(kt == 0), stop=(kt == 1))
                # ---------- final softmax + output ----------
                # zn psum = 32*z; de-scale while adding b2
                pst = spool.tile([CHUNK, 24], F32, tag="pst")
                nc.vector.scalar_tensor_tensor(pst[:], zn_ps[0:CHUNK, :],
                                               1.0 / 32.0, b2c_sb[0:CHUNK, :],
                                               mybir.AluOpType.mult,
                                               mybir.AluOpType.add)
                wst = spool.tile([CHUNK, 20], F32, tag="wst")
                pin = pst[:].rearrange("p (k c) -> p k c", c=6)[:, :, 0:NACT]
                nc.scalar.activation(wst[:].rearrange("p (k a) -> p k a", a=5),
                                     pin, mybir.ActivationFunctionType.Exp)
                s4 = spool.tile([CHUNK, 4], F32, tag="s4")
                nc.vector.tensor_reduce(
                    s4[:], wst[:].rearrange("p (k a) -> p k a", a=5),
                    mybir.AxisListType.X, mybir.AluOpType.add)
                r4 = spool.tile([CHUNK, 4], F32, tag="r4")
                nc.vector.reciprocal(r4[:], s4[:])
                outst = opool.tile([CHUNK, 24], F32, tag="outst")
                oview = outst[:].rearrange("p (k c) -> p k c", c=6)
                nc.vector.tensor_tensor(
                    oview[:, :, 0:NACT],
                    wst[:].rearrange("p (k a) -> p k a", a=5),
                    r4[:].unsqueeze(2).broadcast_to((CHUNK, 4, NACT)),
                    mybir.AluOpType.mult)
                nc.gpsimd.tensor_copy(oview[:, :, NACT:6],
                                      pst[:].rearrange("p (k c) -> p k c", c=6)
                                      [:, :, NACT:6])
                dst = bass.AP(tensor=out_ext[:].tensor, offset=n0 * 6,
                              ap=[[6, CHUNK], [CHUNK * 6, NCHUNK], [1, 6]])
                nc.sync.dma_start(
                    dst, outst[:].rearrange("p (k c) -> p k c", c=6))

            # ---- main pipeline ----
            # iteration i: B(i-6), A1(i), tile A2/A3 (when their last
            # producing super is 2/3 iterations old), A4(i-5)
            load_xt(0)
            if XT_PREF > 1 and nsup > 1:
                load_xt(1)
            from collections import defaultdict
            a2_at = defaultdict(list)
            a3_at = defaultdict(list)
            for t in range(ntiles):
                smax = min(((t + 1) * 126 - 1) // GSUP, nsup - 1)
                a2_at[smax + 1].append(t)
                a3_at[smax + 2].append(t)
            states = {}
            for i in range(nsup + DIST):
                # A-path first so its DMAs sit ahead of B's tail-dependent
                # out-DMA in each queue's FIFO segment for this iteration
                if i < nsup:
                    states[i] = phase_a1(i)
                    states[i]["s"] = i
                for t in a2_at.get(i, []):
                    tile_a2(t)
                for t in a3_at.get(i, []):
                    tile_a3(t)
                if 0 <= i - 4 < nsup:
                    phase_a4(i - 4, states[i - 4])
                if i >= DIST:
                    phase_b(states.pop(i - DIST))

    nc.compile()
    return nc


def _q8(v):
    """Quantize to TRN fp8e4 (max +-240)."""
    return np.clip(np.asarray(v, np.float32), -240.0, 240.0).astype(
        ml_dtypes.float8_e4m3)


def prepare_inputs(x, W_emb, b_emb, W_gat, a_l, a_r, W1, b1, W2, b2,
                   V1, vb1, V2, vb2, nsup=NSUP_FULL):
    """Host-side: shard/pad/transpose x; precompute packed weights."""
    npc = nsup * SUPER
    bf = ml_dtypes.bfloat16
    x = np.asarray(x, np.float32)
    n_real_pc = x.shape[0] // NCORES

    wemb = np.zeros((DOBS, 256), np.float32)
    wemb[:, :D] = np.asarray(W_emb) * 8.0
    Wg = np.asarray(W_gat)

    # wg01: [p, i, 256*h + e] = 32*Wg[h][i*128+p, e]  (h in 0,1)
    wg01 = np.zeros((128, 2, 512), np.float32)
    wg2 = np.zeros((128, 2, 256), np.float32)
    for h in range(2):
        pad = np.zeros((256, D), np.float32)
        pad[:D] = Wg[h] * 32.0
        wg01[:, 0, 256 * h:256 * h + D] = pad[0:128]
        wg01[:, 1, 256 * h:256 * h + D] = pad[128:256]
    pad = np.zeros((256, D), np.float32)
    pad[:D] = Wg[2] * 32.0
    wg2[:, 0, 0:D] = pad[0:128]
    wg2[:, 1, 0:D] = pad[128:256]

    # vlrp: [p, i, r]: r<3 -> 64*vl_r, r in 3..5 -> 64*vr_{r-3}
    vlrp = np.zeros((128, 2, 16), np.float32)
    for h in range(H):
        vl = np.pad(Wg[h] @ np.asarray(a_l)[h] * 64.0, (0, 6))
        vr = np.pad(Wg[h] @ np.asarray(a_r)[h] * 64.0, (0, 6))
        vlrp[:, 0, h] = vl[0:128]
        vlrp[:, 1, h] = vl[128:256]
        vlrp[:, 0, 3 + h] = vr[0:128]
        vlrp[:, 1, 3 + h] = vr[128:256]

    # W1_eff rows: [0:250]=4*W1 top, [256:506]=(8/3)*W1 bottom
    def pack_head1(Wm):
        Wm = np.asarray(Wm)
        eff = np.zeros((512, 256), np.float32)
        eff[0:D] = Wm[0:D] * 4.0
        eff[256:256 + D] = Wm[D:2 * D] * (8.0 / 3.0)
        return np.stack([eff[128 * k:128 * k + 128] for k in range(4)], axis=1)

    w1p = pack_head1(W1)           # [128, 4, 256]
    v1p = pack_head1(V1)
    w2p = np.zeros((128, 2, 8), np.float32)
    w2p[:, 0, 0:NACT] = np.asarray(W2)[0:128]
    w2p[:, 1, 0:NACT] = np.asarray(W2)[128:256]
    v2p = np.zeros((128, 2, 1), np.float32)
    v2p[:, 0, :] = np.asarray(V2)[0:128]
    v2p[:, 1, :] = np.asarray(V2)[128:256]

    biases = np.zeros((128, 8), np.float32)
    be = np.pad(np.asarray(b_emb), (0, 6)) * 8.0
    biases[:, 0] = be[0:128]
    biases[:, 1] = be[128:256]
    biases[:, 2] = np.asarray(b1)[0:128] * 32.0
    biases[:, 3] = np.asarray(b1)[128:256] * 32.0
    biases[:, 4] = np.asarray(vb1)[0:128] * 32.0
    biases[:, 5] = np.asarray(vb1)[128:256] * 32.0
    b2cat = np.zeros((128, 24), np.float32)
    for k in range(4):
        b2cat[:, 6 * k:6 * k + 5] = np.asarray(b2)[None, :]
        b2cat[:, 6 * k + 5] = np.asarray(vb2)[0]
    sidx = np.full((128, 80), -1, np.int16)
    for p in range(CHUNK):
        gg, j = p // 6, p % 6
        for k in range(4):
            for h in range(H):
                for i in range(6):
                    sidx[p, 20 * k + h * 6 + i] = 384 * k + 128 * h + 6 * gg + i

    shared = {
        "wemb": wemb.astype(bf), "wg01": _q8(wg01), "wg2": _q8(wg2),
        "vlrp": _q8(vlrp), "w1p": _q8(w1p), "v1p": _q8(v1p),
        "w2p": w2p.astype(bf), "v2p": v2p.astype(bf),
        "biases": biases, "b2cat": b2cat, "sidx": sidx,
    }
    in_maps = []
    for c in range(NCORES):
        xs = x[c * n_real_pc:(c + 1) * n_real_pc]
        xp = np.zeros((npc + 2, DOBS), np.float32)
        xp[0:min(n_real_pc, npc)] = xs[0:npc]
        m = dict(shared)
        m["xt"] = np.ascontiguousarray(xp.T).astype(bf)
        in_maps.append(m)
    return in_maps, n_real_pc


_BUILD_CACHE = {}


def _get_built(nsup):
    if nsup not in _BUILD_CACHE:
        _BUILD_CACHE[nsup] = build(nsup)
    return _BUILD_CACHE[nsup]


def kernel(**inputs) -> np.ndarray:
    from concourse.bass_utils import run_bass_kernel_spmd
    nc = _get_built(NSUP_FULL)
    in_maps, n_real_pc = prepare_inputs(**inputs)
    res = run_bass_kernel_spmd(nc, in_maps, core_ids=list(range(NCORES)),
                               trace=False)
    outs = [res.results[c]["out"][0:n_real_pc] for c in range(NCORES)]
    return np.concatenate(outs, axis=0).astype(np.float32)


# revision 29
# speedup vs baseline: 1.0236x; 1.0236x over previous
"""Trainium2 Bass kernel for nn_ActorCritic (GAT actor-critic over B=16384
fully-connected 6-node graphs), data-parallel over 8 NeuronCores.

v3: fp8 DoubleRow (K=256/pass) for the Wh and head GEMMs; embed / zn /
aggregation stay bf16 (fp8 there costs accuracy and buys no PE time).
phase_a is split into 4 pipeline stages so every DMA's producer finished
>=1 super earlier -- the DRAM round trips (elr transpose, attention
scatter) no longer stall the DMA queues.

Self-contained: hardcodes all shapes; imports only the system concourse repo.
"""
import sys

if "/opt/trn_rl_repo" not in sys.path:
    sys.path.insert(0, "/opt/trn_rl_repo")

import numpy as np
import ml_dtypes

import concourse.bass as bass
import concourse.mybir as mybir
import concourse.tile as tile
from concourse import bacc

BF16 = mybir.dt.bfloat16
F32 = mybir.dt.float32
FP8 = mybir.dt.float8e4
I16 = mybir.dt.int16
DR = mybir.MatmulPerfMode.DoubleRow

# problem constants
B, A, DOBS, D, H, HID, NACT = 16384, 6, 128, 250, 3, 256, 5
SLOPE = 0.2
NCORES = 8
G_PC = B // NCORES            # 2048 real graphs / core
GSUP = 84                     # graphs per super-block
SUPER = GSUP * A              # 504 nodes per super-block
NSUP_FULL = 25                # supers per core (pad 2048 -> 2100 graphs)
CHUNK = 126                   # nodes per aggregation chunk (21 graphs)
NCHUNK = SUPER // CHUNK       # 4
DIST = 6                      # pipeline distance between stage A1 and B
XT_PREF = 2                   # xt prefetch distance (supers)

# scale plan (stored = scale * true):
#   x: 1 (bf16)   wemb: 8 (bf16)   hT = 8*h (fp8)   wg: 32 (fp8)
#   Wh psum / wsb bf16 = 256*Wh    at4 = attn (bf16)
#   gat psum = 256*gat_sum         feat gat subtiles = 4*gat_sum (fp8)
#   W1 top *4, W1 bot *(8/3) (fp8) -> heads psum = 32*y1
#   a1 = 32*relu (bf16, bias*32)   w2 raw (bf16) -> zn psum = 32*z
#   vlr: 64 (fp8) -> elr psum = 512*elr (de-scaled in the exp)


def build(nsup=NSUP_FULL):
    """Trace + compile the per-core Bass graph. Returns compiled Bacc."""
    npc = nsup * SUPER  # padded nodes per core
    nc = bacc.Bacc("TRN2", target_bir_lowering=False, debug=False,
                   num_devices=NCORES)

    # ---- DRAM parameters ----
    xt_ext = nc.dram_tensor("xt", [DOBS, npc + 2], BF16, kind="ExternalInput")
    wemb_ext = nc.dram_tensor("wemb", [DOBS, 256], BF16, kind="ExternalInput")
    wg01_ext = nc.dram_tensor("wg01", [128, 2, 512], FP8, kind="ExternalInput")
    wg2_ext = nc.dram_tensor("wg2", [128, 2, 256], FP8, kind="ExternalInput")
    vlrp_ext = nc.dram_tensor("vlrp", [128, 2, 16], FP8, kind="ExternalInput")
    w1p_ext = nc.dram_tensor("w1p", [128, 4, 256], FP8, kind="ExternalInput")
    v1p_ext = nc.dram_tensor("v1p", [128, 4, 256], FP8, kind="ExternalInput")
    w2p_ext = nc.dram_tensor("w2p", [128, 2, 8], BF16, kind="ExternalInput")
    v2p_ext = nc.dram_tensor("v2p", [128, 2, 1], BF16, kind="ExternalInput")
    bias_ext = nc.dram_tensor("biases", [128, 8], F32, kind="ExternalInput")
    b2c_ext = nc.dram_tensor("b2cat", [128, 24], F32, kind="ExternalInput")
    sidx_ext = nc.dram_tensor("sidx", [128, 80], I16, kind="ExternalInput")
    out_ext = nc.dram_tensor("out", [npc, 6], F32, kind="ExternalOutput")

    # DRAM scratch for cross-layout reshapes. Global layouts: the attention
    # stats stages run on 126-graph tiles (full lanes) decoupled from the
    # 84-graph super cadence; DRAM is the granularity adapter.
    ntiles = (nsup * GSUP + 125) // 126
    ngpad = ntiles * 126
    elrd = nc.dram_tensor("elrd", [6, ngpad * A], F32)
    natd = nc.dram_tensor("natd", [ngpad, 108], BF16)

    with tile.TileContext(nc) as tc:
        # ---- persistent SBUF: weights/constants ----
        wemb_sb = nc.alloc_sbuf_tensor("wemb_sb", [DOBS, 256], BF16)
        wg01_sb = nc.alloc_sbuf_tensor("wg01_sb", [128, 2, 512], FP8)
        wg2_sb = nc.alloc_sbuf_tensor("wg2_sb", [128, 2, 256], FP8)
        vlrp_sb = nc.alloc_sbuf_tensor("vlrp_sb", [128, 2, 16], FP8)
        w1p_sb = nc.alloc_sbuf_tensor("w1p_sb", [128, 4, 256], FP8)
        v1p_sb = nc.alloc_sbuf_tensor("v1p_sb", [128, 4, 256], FP8)
        w2p_sb = nc.alloc_sbuf_tensor("w2p_sb", [128, 2, 8], BF16)
        v2p_sb = nc.alloc_sbuf_tensor("v2p_sb", [128, 2, 1], BF16)
        bias_sb = nc.alloc_sbuf_tensor("bias_sb", [128, 8], F32)
        b2c_sb = nc.alloc_sbuf_tensor("b2c_sb", [128, 24], F32)
        sidx_sb = nc.alloc_sbuf_tensor("sidx_sb", [128, 80], I16)

        nc.scalar.dma_start(wemb_sb[:], wemb_ext[:])
        nc.scalar.dma_start(bias_sb[:], bias_ext[:])
        nc.scalar.dma_start(vlrp_sb[:], vlrp_ext[:])
        nc.scalar.dma_start(sidx_sb[:], sidx_ext[:])

        deferred = [(wg01_sb[:], wg01_ext[:]), (wg2_sb[:], wg2_ext[:]),
                    (w1p_sb[:], w1p_ext[:]), (v1p_sb[:], v1p_ext[:]),
                    (w2p_sb[:], w2p_ext[:]), (v2p_sb[:], v2p_ext[:]),
                    (b2c_sb[:], b2c_ext[:])]
        _pump_i = [0]

        def pump_preloads(n):
            engines = [nc.sync, nc.scalar, nc.gpsimd]
            for _ in range(n):
                if not deferred:
                    return
                dst, srcd = deferred.pop(0)
                engines[_pump_i[0] % 3].dma_start(dst, srcd)
                _pump_i[0] += 1

        # ---- pools ----
        import contextlib
        ctx = contextlib.ExitStack()
        with ctx:
            xpool = ctx.enter_context(tc.tile_pool(name="xp", bufs=4))
            hpool = ctx.enter_context(tc.tile_pool(name="hp", bufs=DIST + 1))
            whpool = ctx.enter_context(tc.tile_pool(name="whp", bufs=4))
            a1pool = ctx.enter_context(tc.tile_pool(name="a1p", bufs=8))
            apool = ctx.enter_context(tc.tile_pool(name="ap", bufs=DIST + 1))
            spool = ctx.enter_context(tc.tile_pool(name="sp", bufs=4))
            opool = ctx.enter_context(tc.tile_pool(name="op", bufs=3))
            # PSUM: 8 banks total; tags: emb 2 + elr 1 + big2 3 + gat 2
            ppool = ctx.enter_context(tc.tile_pool(name="pp", bufs=2, space="PSUM"))

            xt_tiles = {}

            def load_xt(s):
                n0 = s * SUPER
                xt = xpool.tile([DOBS, SUPER + 2], BF16, tag="xt")
                nc.sync.dma_start(xt[:], xt_ext[:, n0:n0 + SUPER + 2])
                xt_tiles[s] = xt

            def phase_a1(s):
                """embed + elr matmul + elr -> DRAM."""
                if s + XT_PREF < nsup:
                    load_xt(s + XT_PREF)
                xt = xt_tiles.pop(s)
                h_ps0 = ppool.tile([128, SUPER + 2], F32, tag="emb", bufs=2)
                h_ps1 = ppool.tile([128, SUPER + 2], F32, tag="emb", bufs=2)
                nc.tensor.matmul(h_ps0[:], wemb_sb[:, 0:128], xt[:],
                                 start=True, stop=True)
                nc.tensor.matmul(h_ps1[:], wemb_sb[:, 128:256], xt[:],
                                 start=True, stop=True)
                # hT: fp8 feat tile; subtiles 0,1 = 8*h halves (2,3 = gat later)
                hT = hpool.tile([128, 4, 512], FP8, tag="hT")
                nc.scalar.activation(
                    hT[:, 0, 0:506], h_ps0[:, 0:506],
                    mybir.ActivationFunctionType.Relu,
                    bias=bias_sb[:, 0:1])
                nc.scalar.activation(
                    hT[:, 1, 0:506], h_ps1[:, 0:506],
                    mybir.ActivationFunctionType.Relu,
                    bias=bias_sb[:, 1:2])
                # el/er (fp8 DoubleRow, K=256 in one pass)
                elr_ps = ppool.tile([6, SUPER + 2], F32, tag="gat", bufs=2)
                nc.tensor.matmul(elr_ps[:, 0:SUPER], vlrp_sb[:, :, 0:6],
                                 hT[:, 0:2, 0:SUPER],
                                 start=True, stop=True, perf_mode=DR)
                elr_sb = spool.tile([6, SUPER], F32, tag="elr_sb")
                nc.scalar.copy(elr_sb[:], elr_ps[:, 0:SUPER])
                nc.scalar.dma_start(elrd[:, s * SUPER:(s + 1) * SUPER], elr_sb[:])
                pump_preloads(5)
                return {"hT": hT}

            tstates = {}

            def tile_a2(t):
                """read elr back transposed to graph-major, 126-graph tile."""
                elrg = spool.tile([126, 36], F32, tag="elrg")
                src = bass.AP(tensor=elrd[:].tensor, offset=t * 126 * 6,
                              ap=[[6, 126], [ngpad * A, 6], [1, 6]])
                nc.sync.dma_start(elrg[:].rearrange("p (r i) -> p r i", i=6), src)
                tstates[t] = elrg

            def tile_a3(t):
                """attention stats (graph-major, 126 lanes) + natd -> DRAM."""
                elrg = tstates.pop(t)
                e_t = spool.tile([126, 108], F32, tag="e_t")
                in0 = (elrg[:, 0:18].rearrange("p (h i) -> p h i", i=6)
                       .unsqueeze(3).broadcast_to((126, 3, 6, 6)))
                in1 = (elrg[:, 18:36].rearrange("p (h j) -> p h j", j=6)
                       .unsqueeze(2).broadcast_to((126, 3, 6, 6)))
                ev = e_t[:].rearrange("p (h i j) -> p h i j", i=6, j=6)
                nc.vector.tensor_tensor(ev, in0, in1, mybir.AluOpType.add)
                w_t = spool.tile([126, 108], F32, tag="w_t")
                lk = spool.tile([126, 108], F32, tag="lk")
                nc.vector.scalar_tensor_tensor(lk[:], e_t[:], SLOPE, e_t[:],
                                               mybir.AluOpType.mult,
                                               mybir.AluOpType.max)
                # elr psum carries 512x scale; fold 1/512 into the exp
                nc.scalar.activation(w_t[:], lk[:],
                                     mybir.ActivationFunctionType.Exp,
                                     scale=1.0 / 512.0)
                s_t = spool.tile([126, 18], F32, tag="s_t")
                nc.vector.tensor_reduce(
                    s_t[:], w_t[:].rearrange("p (hi j) -> p hi j", j=6),
                    mybir.AxisListType.X, mybir.AluOpType.add)
                r_t = spool.tile([126, 18], F32, tag="r_t")
                nc.vector.reciprocal(r_t[:], s_t[:])
                natg = spool.tile([126, 112], BF16, tag="natg")
                nout = natg[:, 0:108].rearrange("p (j h i) -> p h i j", h=3, i=6)
                nin0 = w_t[:].rearrange("p (h i j) -> p h i j", i=6, j=6)
                nin1 = (r_t[:].rearrange("p (h i) -> p h i", i=6)
                        .unsqueeze(3).broadcast_to((126, 3, 6, 6)))
                nc.vector.tensor_tensor(nout, nin0, nin1, mybir.AluOpType.mult)
                nc.gpsimd.dma_start(natd[t * 126:(t + 1) * 126], natg[:, 0:108])

            def phase_a4(s, st):
                """node-major gather + block-diag scatter."""
                dataN4 = spool.tile([128, 80], BF16, tag="dataN")
                engs = [nc.sync, nc.gpsimd, nc.sync, nc.scalar]
                for k in range(NCHUNK):
                    srcn = bass.AP(tensor=natd[:].tensor,
                                   offset=(s * GSUP + 21 * k) * 108,
                                   ap=[[108, 21], [18, 6], [1, 18]])
                    engs[k].dma_start(dataN4[0:CHUNK, 20 * k:20 * k + 18], srcn)
                at4 = apool.tile([128, 4 * 384], BF16, tag="atall")
                nc.gpsimd.local_scatter(at4[:], dataN4[:], sidx_sb[:],
                                        channels=128, num_elems=4 * 384,
                                        num_idxs=80)
                st["at4"] = at4

            def phase_b(st):
                hT, at4 = st["hT"], st["at4"]
                s = st["s"]
                n0 = s * SUPER
                # ---------- per-chunk: Wh (fp8 DoubleRow) + aggregation ----
                gatL = ppool.tile([128, SUPER], F32, tag="gat", bufs=2)
                gatU = ppool.tile([128, SUPER], F32, tag="gat", bufs=2)
                wh_tiles = [None] * NCHUNK

                def wh_chunk(k):
                    c0 = k * CHUNK
                    whA = ppool.tile([126, 512], F32, tag="big2", bufs=4)
                    nc.tensor.matmul(whA[:], hT[:, 0:2, c0:c0 + CHUNK],
                                     wg01_sb[:], start=True, stop=True,
                                     perf_mode=DR)
                    whB = ppool.tile([126, 256], F32, tag="big2", bufs=4)
                    nc.tensor.matmul(whB[:], hT[:, 0:2, c0:c0 + CHUNK],
                                     wg2_sb[:], start=True, stop=True,
                                     perf_mode=DR)
                    wsb01 = whpool.tile([126, 512], BF16, tag="wh01")
                    nc.vector.tensor_copy(wsb01[:], whA[:])
                    wsb2 = whpool.tile([126, 256], BF16, tag="wh2")
                    nc.vector.tensor_copy(wsb2[:], whB[:])
                    wh_tiles[k] = (wsb01, wsb2)

                def gat_chunk(k):
                    c0 = k * CHUNK
                    wsb01, wsb2 = wh_tiles[k]
                    # hi halves carry 6 zero pad cols so M=128 everywhere
                    wh_t = [(wsb01, 0, 128), (wsb01, 256, 384), (wsb2, 0, 128)]
                    for h in range(H):
                        tl, lo, _ = wh_t[h]
                        nc.tensor.matmul(
                            gatL[:, c0:c0 + CHUNK],
                            tl[0:CHUNK, lo:lo + 128],
                            at4[0:CHUNK, 384 * k + 128 * h:384 * k + 128 * h + CHUNK],
                            start=(h == 0), stop=(h == 2))
                    for h in range(H):
                        tl, _, hi = wh_t[h]
                        nc.tensor.matmul(
                            gatU[:, c0:c0 + CHUNK],
                            tl[0:CHUNK, hi:hi + 128],
                            at4[0:CHUNK, 384 * k + 128 * h:384 * k + 128 * h + CHUNK],
                            start=(h == 0), stop=(h == 2))

                wh_chunk(0)
                wh_chunk(1)
                gat_chunk(0)
                wh_chunk(2)
                gat_chunk(1)
                wh_chunk(3)
                gat_chunk(2)
                gat_chunk(3)
                # gat psum = 256*gat_sum; store 4*gat_sum into feat subtiles
                nc.vector.tensor_scalar(hT[:, 2, 0:SUPER], gatL[:, 0:SUPER],
                                        1.0 / 64.0, 0.0,
                                        mybir.AluOpType.mult,
                                        mybir.AluOpType.add)
                nc.scalar.activation(hT[:, 3, 0:SUPER], gatU[:, 0:SUPER],
                                     mybir.ActivationFunctionType.Copy,
                                     scale=1.0 / 64.0)

                # ---------- heads (fp8 DoubleRow, 2 K-passes of 256) -------
                a1_sb, c1_sb = [], []
                for which, (wp, bcol, dst) in enumerate(
                        [(w1p_sb, 2, a1_sb), (v1p_sb, 4, c1_sb)]):
                    for m in range(2):
                        ps = ppool.tile([128, SUPER], F32, tag="big2", bufs=4)
                        nc.tensor.matmul(ps[:],
                                         wp[:, 0:2, 128 * m:128 * m + 128],
                                         hT[:, 0:2, 0:SUPER],
                                         start=True, stop=False, perf_mode=DR)
                        nc.tensor.matmul(ps[:],
                                         wp[:, 2:4, 128 * m:128 * m + 128],
                                         hT[:, 2:4, 0:SUPER],
                                         start=False, stop=True, perf_mode=DR)
                        sb = a1pool.tile([128, SUPER + 4], BF16, tag="a1")
                        bap = bias_sb[:, bcol + m:bcol + m + 1]
                        if which == 0:
                            nc.vector.tensor_scalar(
                                sb[:, 0:SUPER], ps[:], bap, 0.0,
                                mybir.AluOpType.add, mybir.AluOpType.max)
                        else:
                            nc.scalar.activation(
                                sb[:, 0:SUPER], ps[:],
                                mybir.ActivationFunctionType.Relu, bias=bap)
                        dst.append(sb)
                # ---------- zn (bf16) ----------
                zn_ps = ppool.tile([128, 24], F32, tag="gat", bufs=2)
                for k in range(NCHUNK):
                    c0 = k * CHUNK
                    for kt in range(2):
                        nc.tensor.matmul(zn_ps[:, 6 * k:6 * k + NACT],
                                         a1_sb[kt][:, c0:c0 + 128],
                                         w2p_sb[:, kt, 0:NACT],
                                         start=(kt == 0), stop=(kt == 1))
                    for kt in range(2):
                        nc.tensor.matmul(zn_ps[:, 6 * k + NACT:6 * k + 6],
                                         c1_sb[kt][:, c0:c0 + 128],
                                         v2p_sb[:, kt, :],
                                         start=(kt == 0), stop=(kt == 1))
                # ---------- final softmax + output ----------
                # zn psum = 32*z; de-scale while adding b2
                pst = spool.tile([CHUNK, 24], F32, tag="pst")
                nc.vector.scalar_tensor_tensor(pst[:], zn_ps[0:CHUNK, :],
                                               1.0 / 32.0, b2c_sb[0:CHUNK, :],
                                               mybir.AluOpType.mult,
                                               mybir.AluOpType.add)
                wst = spool.tile([CHUNK, 20], F32, tag="wst")
                pin = pst[:].rearrange("p (k c) -> p k c", c=6)[:, :, 0:NACT]
                nc.scalar.activation(wst[:].rearrange("p (k a) -> p k a", a=5),
                                     pin, mybir.ActivationFunctionType.Exp)
                s4 = spool.tile([CHUNK, 4], F32, tag="s4")
                nc.vector.tensor_reduce(
                    s4[:], wst[:].rearrange("p (k a) -> p k a", a=5),
                    mybir.AxisListType.X, mybir.AluOpType.add)
                r4 = spool.tile([CHUNK, 4], F32, tag="r4")
                nc.vector.reciprocal(r4[:], s4[:])
                outst = opool.tile([CHUNK, 24], F32, tag="outst")
                oview = outst[:].rearrange("p (k c) -> p k c", c=6)
                nc.vector.tensor_tensor(
                    oview[:, :, 0:NACT],
                    wst[:].rearrange("p (k a) -> p k a", a=5),
                    r4[:].unsqueeze(2).broadcast_to((CHUNK, 4, NACT)),
                    mybir.AluOpType.mult)
                nc.gpsimd.tensor_copy(oview[:, :, NACT:6],
                                      pst[:].rearrange("p (k c) -> p k c", c=6)
                                      [:, :, NACT:6])
                dst = bass.AP(tensor=out_ext[:].tensor, offset=n0 * 6,
                              ap=[[6, CHUNK], [CHUNK * 6, NCHUNK], [1, 6]])
                nc.scalar.dma_start(
                    dst, outst[:].rearrange("p (k c) -> p k c", c=6))

            # ---- main pipeline ----
            # iteration i: B(i-6), A1(i), tile A2/A3 (when their last
            # producing super is 2/3 iterations old), A4(i-5)
            load_xt(0)
            if XT_PREF > 1 and nsup > 1:
                load_xt(1)
            from collections import defaultdict
            a2_at = defaultdict(list)
            a3_at = defaultdict(list)
            for t in range(ntiles):
                smax = min(((t + 1) * 126 - 1) // GSUP, nsup - 1)
                a2_at[smax + 1].append(t)
                a3_at[smax + 2].append(t)
            states = {}
            for i in range(nsup + DIST):
                if i >= DIST:
                    phase_b(states.pop(i - DIST))
                if i < nsup:
                    states[i] = phase_a1(i)
                    states[i]["s"] = i
                for t in a2_at.get(i, []):
                    tile_a2(t)
                for t in a3_at.get(i, []):
                    tile_a3(t)
                if 0 <= i - 4 < nsup:
                    phase_a4(i - 4, states[i - 4])

    nc.compile()
    return nc


def _q8(v):
    """Quantize to TRN fp8e4 (max +-240)."""
    return np.clip(np.asarray(v, np.float32), -240.0, 240.0).astype(
        ml_dtypes.float8_e4m3)


def prepare_inputs(x, W_emb, b_emb, W_gat, a_l, a_r, W1, b1, W2, b2,
                   V1, vb1, V2, vb2, nsup=NSUP_FULL):
    """Host-side: shard/pad/transpose x; precompute packed weights."""
    npc = nsup * SUPER
    bf = ml_dtypes.bfloat16
    x = np.asarray(x, np.float32)
    n_real_pc = x.shape[0] // NCORES

    wemb = np.zeros((DOBS, 256), np.float32)
    wemb[:, :D] = np.asarray(W_emb) * 8.0
    Wg = np.asarray(W_gat)

    # wg01: [p, i, 256*h + e] = 32*Wg[h][i*128+p, e]  (h in 0,1)
    wg01 = np.zeros((128, 2, 512), np.float32)
    wg2 = np.zeros((128, 2, 256), np.float32)
    for h in range(2):
        pad = np.zeros((256, D), np.float32)
        pad[:D] = Wg[h] * 32.0
        wg01[:, 0, 256 * h:256 * h + D] = pad[0:128]
        wg01[:, 1, 256 * h:256 * h + D] = pad[128:256]
    pad = np.zeros((256, D), np.float32)
    pad[:D] = Wg[2] * 32.0
    wg2[:, 0, 0:D] = pad[0:128]
    wg2[:, 1, 0:D] = pad[128:256]

    # vlrp: [p, i, r]: r<3 -> 64*vl_r, r in 3..5 -> 64*vr_{r-3}
    vlrp = np.zeros((128, 2, 16), np.float32)
    for h in range(H):
        vl = np.pad(Wg[h] @ np.asarray(a_l)[h] * 64.0, (0, 6))
        vr = np.pad(Wg[h] @ np.asarray(a_r)[h] * 64.0, (0, 6))
        vlrp[:, 0, h] = vl[0:128]
        vlrp[:, 1, h] = vl[128:256]
        vlrp[:, 0, 3 + h] = vr[0:128]
        vlrp[:, 1, 3 + h] = vr[128:256]

    # W1_eff rows: [0:250]=4*W1 top, [256:506]=(8/3)*W1 bottom
    def pack_head1(Wm):
        Wm = np.asarray(Wm)
        eff = np.zeros((512, 256), np.float32)
        eff[0:D] = Wm[0:D] * 4.0
        eff[256:256 + D] = Wm[D:2 * D] * (8.0 / 3.0)
        return np.stack([eff[128 * k:128 * k + 128] for k in range(4)], axis=1)

    w1p = pack_head1(W1)           # [128, 4, 256]
    v1p = pack_head1(V1)
    w2p = np.zeros((128, 2, 8), np.float32)
    w2p[:, 0, 0:NACT] = np.asarray(W2)[0:128]
    w2p[:, 1, 0:NACT] = np.asarray(W2)[128:256]
    v2p = np.zeros((128, 2, 1), np.float32)
    v2p[:, 0, :] = np.asarray(V2)[0:128]
    v2p[:, 1, :] = np.asarray(V2)[128:256]

    biases = np.zeros((128, 8), np.float32)
    be = np.pad(np.asarray(b_emb), (0, 6)) * 8.0
    biases[:, 0] = be[0:128]
    biases[:, 1] = be[128:256]
    biases[:, 2] = np.asarray(b1)[0:128] * 32.0
    biases[:, 3] = np.asarray(b1)[128:256] * 32.0
    biases[:, 4] = np.asarray(vb1)[0:128] * 32.0
    biases[:, 5] = np.asarray(vb1)[128:256] * 32.0
    b2cat = np.zeros((128, 24), np.float32)
    for k in range(4):
        b2cat[:, 6 * k:6 * k + 5] = np.asarray(b2)[None, :]
        b2cat[:, 6 * k + 5] = np.asarray(vb2)[0]
    sidx = np.full((128, 80), -1, np.int16)
    for p in range(CHUNK):
        gg, j = p // 6, p % 6
        for k in range(4):
            for h in range(H):
                for i in range(6):
                    sidx[p, 20 * k + h * 6 + i] = 384 * k + 128 * h + 6 * gg + i

    shared = {
        "wemb": wemb.astype(bf), "wg01": _q8(wg01), "wg2": _q8(wg2),
        "vlrp": _q8(vlrp), "w1p": _q8(w1p), "v1p": _q8(v1p),
        "w2p": w2p.astype(bf), "v2p": v2p.astype(bf),
        "biases": biases, "b2cat": b2cat, "sidx": sidx,
    }
    in_maps = []
    for c in range(NCORES):
        xs = x[c * n_real_pc:(c + 1) * n_real_pc]
        xp = np.zeros((npc + 2, DOBS), np.float32)
        xp[0:min(n_real_pc, npc)] = xs[0:npc]
        m = dict(shared)
        m["xt"] = np.ascontiguousarray(xp.T).astype(bf)
        in_maps.append(m)
    return in_maps, n_real_pc


_BUILD_CACHE = {}


def _get_built(nsup):
    if nsup not in _BUILD_CACHE:
        _BUILD_CACHE[nsup] = build(nsup)
    return _BUILD_CACHE[nsup]


def kernel(**inputs) -> np.ndarray:
    from concourse.bass_utils import run_bass_kernel_spmd
    nc = _get_built(NSUP_FULL)
    in_maps, n_real_pc = prepare_inputs(**inputs)
    res = run_bass_kernel_spmd(nc, in_maps, core_ids=list(range(NCORES)),
                               trace=False)
    outs = [res.results[c]["out"][0:n_real_pc] for c in range(NCORES)]
    return np.concatenate(outs, axis=0).astype(np.float32)
